# revision 25
# baseline (speedup 1.0000x reference)
"""DKVB vq_codebook kernel for 8 Trainium2 NeuronCores.

Strategy (sharding_hint): shard the C=256 codebooks across 8 cores (32 each).
Each core computes, for its codebooks c and all 256 tokens t:
    xp   = X @ P_c                      (projection,  fp16 hi/lo 3-term on PE)
    s    = xp @ cb_c^T - 0.5*||cb_c||^2 (score; argmax s == argmin d2)
    idx  = argmax_m s                   (DVE max8 + max_index, exact fp32)
    out += values_c[idx]                (HBM dma_gather by row)
Host sums the 8 partial [256,10] results and divides by 256.

Numerics: fp16 hi/lo splits make every matmul term exact-to-~2^-22; the
score is assembled in fp32 PSUM, so the argmax matches the fp32 reference
(0 flips verified on the actual seed-0 data vs fp64 ground truth).

All tensors are host-pre-laid-out so every big DMA is partition-contiguous.
"""

import sys
import numpy as np

sys.path.insert(0, "/opt/trn_rl_repo")

B, N, D = 4, 64, 2048
C, M, E, V = 256, 4096, 128, 10
NCORES = 8
CLOC = C // NCORES          # codebooks per core
T = B * N                   # 256 tokens
KCH = D // 128              # 16 k-chunks for the projection matmul
MCH = M // 512              # 8 m-chunks of 512 for the score matmul
S1, S2, S3 = 32.0, 512.0, 128.0   # X scale, P scale, xp rescale target
VPAD = 64                   # values rows padded to 64 fp32 = 256B (dma_gather)

_CACHE = {}


# --------------------------------------------------------------------------
# device program
# --------------------------------------------------------------------------
def build_nc(cloc=CLOC, debug_out=False):
    import concourse.bacc as bacc
    import concourse.bass as bass
    import concourse.tile as tile
    from concourse import mybir
    from contextlib import ExitStack

    f16 = mybir.dt.float16
    f32 = mybir.dt.float32
    u16 = mybir.dt.uint16
    AF = mybir.ActivationFunctionType

    nc = bacc.Bacc("TRN2", target_bir_lowering=False, debug=True)

    # ---- I/O ----
    xh_d = nc.dram_tensor("xh", [128, KCH * T], f16, kind="ExternalInput")
    xl_d = nc.dram_tensor("xl", [128, KCH * T], f16, kind="ExternalInput")
    ph_d = nc.dram_tensor("ph", [cloc, 128, KCH * 128], f16, kind="ExternalInput")
    pl_d = nc.dram_tensor("pl", [cloc, 128, KCH * 128], f16, kind="ExternalInput")
    ch_d = nc.dram_tensor("ch", [cloc, 128, M], f16, kind="ExternalInput")
    cl_d = nc.dram_tensor("cl", [cloc, 128, M], f16, kind="ExternalInput")
    eq_d = nc.dram_tensor("eq", [cloc, 2, M], f16, kind="ExternalInput")
    vl_d = nc.dram_tensor("vl", [cloc * M, VPAD], f32, kind="ExternalInput")

    acc_d = nc.dram_tensor("acc", [128, 2 * VPAD], f32, kind="ExternalOutput")
    idx_d = nc.dram_tensor("idx", [128, 2 * cloc], u16, kind="ExternalOutput")
    if debug_out:
        dwr_d = nc.dram_tensor("dbg_wr", [128, 16], u16, kind="ExternalOutput")
        dg_d = nc.dram_tensor("dbg_g", [128, 2 * VPAD], f32, kind="ExternalOutput")

    with tile.TileContext(nc) as tc, ExitStack() as ctx:
        p_x = ctx.enter_context(tc.tile_pool(name="x", bufs=1))
        p_p = ctx.enter_context(tc.tile_pool(name="p", bufs=2))
        p_cb = ctx.enter_context(tc.tile_pool(name="cb", bufs=2))
        p_eq = ctx.enter_context(tc.tile_pool(name="eq", bufs=2))
        p_xp16 = ctx.enter_context(tc.tile_pool(name="xp16", bufs=2))
        p_score = ctx.enter_context(tc.tile_pool(name="score", bufs=3))
        p_mx = ctx.enter_context(tc.tile_pool(name="mx", bufs=4))
        p_misc = ctx.enter_context(tc.tile_pool(name="misc", bufs=1))
        p_g = ctx.enter_context(tc.tile_pool(name="g", bufs=2))
        p_psxp = ctx.enter_context(tc.tile_pool(name="psxp", bufs=2, space="PSUM"))
        p_psdot = ctx.enter_context(tc.tile_pool(name="psdot", bufs=6, space="PSUM"))
        p_dram = ctx.enter_context(tc.tile_pool(name="scratch", bufs=2, space="DRAM"))

        # ---- static tiles ----
        x_h = p_x.tile([128, KCH * T], f16, tag="xh")
        x_l = p_x.tile([128, KCH * T], f16, tag="xl")
        nc.sync.dma_start(x_h[:], xh_d[:])
        nc.sync.dma_start(x_l[:], xl_d[:])

        ones16 = p_misc.tile([2, 128], f16, tag="ones")
        nc.vector.memset(ones16[:], 1.0)

        idxall = p_misc.tile([128, 8, 2 * cloc], u16, tag="idxall")
        acc = p_misc.tile([128, 2, VPAD], f32, tag="acc")
        nc.vector.memset(acc[:], 0.0)

        def load_cb_weights(c):
            p_h = p_p.tile([128, KCH * 128], f16, tag="ph")
            p_l = p_p.tile([128, KCH * 128], f16, tag="pl")
            nc.sync.dma_start(p_h[:], ph_d[c])
            nc.sync.dma_start(p_l[:], pl_d[c])
            return p_h, p_l

        def compute_xp(p_h, p_l):
            """projection xp_ps[e, t] = sum_d P[d,e] * X[d,t] (scaled), then
            split to an fp16 pair at scale S3."""
            xp_ps = p_psxp.tile([128, T], f32, tag="xp")
            n3 = 3 * KCH
            i = 0
            for k in range(KCH):
                lw_h = p_h[:, k * 128:(k + 1) * 128]
                lw_l = p_l[:, k * 128:(k + 1) * 128]
                rh_h = x_h[:, k * T:(k + 1) * T]
                rh_l = x_l[:, k * T:(k + 1) * T]
                nc.tensor.matmul(xp_ps[:], lw_h, rh_h, start=(i == 0), stop=False)
                i += 1
                nc.tensor.matmul(xp_ps[:], lw_h, rh_l, start=False, stop=False)
                i += 1
                nc.tensor.matmul(xp_ps[:], lw_l, rh_h, start=False, stop=(i == n3 - 1))
                i += 1
            sc = float(S3 / (S1 * S2))
            xh16 = p_xp16.tile([128, T], f16, tag="xh16")
            xp32 = p_xp16.tile([128, T], f32, tag="xp32")
            nc.scalar.activation(xh16[:], xp_ps[:], AF.Copy, scale=sc)
            nc.scalar.activation(xp32[:], xp_ps[:], AF.Copy, scale=sc)
            xl16 = p_xp16.tile([128, T], f16, tag="xl16")
            nc.vector.tensor_sub(xl16[:], xp32[:], xh16[:])
            return xh16, xl16

        pw = load_cb_weights(0)
        xpair = compute_xp(*pw)
        for c in range(cloc):
            # ---- load per-codebook tables ----
            cb_h = p_cb.tile([128, M], f16, tag="ch")
            cb_l = p_cb.tile([128, M], f16, tag="cl")
            nc.sync.dma_start(cb_h[:], ch_d[c])
            nc.sync.dma_start(cb_l[:], cl_d[c])
            eq_t = p_eq.tile([2, M], f16, tag="eq")
            nc.sync.dma_start(eq_t[:], eq_d[c])
            xh16, xl16 = xpair

            # ---- score matmuls + scan per token-chunk ----
            # weights-major emission within each m-half so the PE reuses the
            # stationary operand across consecutive matmuls.
            for tcn in range(2):
                lw_xh = xh16[:, tcn * 128:(tcn + 1) * 128]
                lw_xl = xl16[:, tcn * 128:(tcn + 1) * 128]
                score = p_score.tile([128, M], f32, tag="score")
                for half in range(2):
                    tiles = []
                    for q in range(MCH // 2):
                        mc = half * (MCH // 2) + q
                        ds = p_psdot.tile([128, 512], f32, tag="ds")
                        tiles.append((mc, ds, slice(mc * 512, (mc + 1) * 512)))
                    for mc, ds, ms in tiles:
                        nc.tensor.matmul(ds[:], lw_xh, cb_h[:, ms], start=True, stop=False)
                    for mc, ds, ms in tiles:
                        nc.tensor.matmul(ds[:], lw_xh, cb_l[:, ms], start=False, stop=False)
                    for mc, ds, ms in tiles:
                        nc.tensor.matmul(ds[:], lw_xl, cb_h[:, ms], start=False, stop=False)
                    for mc, ds, ms in tiles:
                        nc.tensor.matmul(ds[:], ones16[:], eq_t[:, ms], start=False, stop=True)
                    for mc, ds, ms in tiles:
                        nc.scalar.activation(score[:, ms], ds[:], AF.Copy)

                mx = p_mx.tile([128, 8], f32, tag="mx")
                nc.vector.max(mx[:], score[:])
                nc.vector.max_index(idxall[:, :, 2 * c + tcn], mx[:], score[:])

                # software pipeline: give the PE the next codebook's
                # projection while ScalarE/DVE digest this token-chunk.
                if tcn == 0 and c + 1 < cloc:
                    pw = load_cb_weights(c + 1)
                    xpair_next = compute_xp(*pw)

            if c + 1 < cloc:
                xpair = xpair_next

            # ---- idx round-trip through DRAM to the wrapped gather layout ----
            # gather slot i = j*16 + q with j = s*2 + tc reads token
            # t = tc*128 + q*8 + s; gathered row i lands on out partition i%128.
            idq = p_dram.tile([128, 2], u16, tag="idq")
            nc.sync.dma_start(idq[:], idxall[:, 0, 2 * c:2 * c + 2])
            wrapped = p_g.tile([128, 16], u16, tag="wrapped")
            src = idq[:].rearrange("(q s) tc -> q s tc", s=8)
            for g in range(8):
                nc.sync.dma_start(wrapped[16 * g:16 * (g + 1), :], src)
            gt = p_g.tile([128, 2, VPAD], f32, tag="g")
            nc.gpsimd.dma_gather(
                gt[:],
                vl_d[c * M:(c + 1) * M, :],
                wrapped[:].bitcast(mybir.dt.int16),
                num_idxs=T,
                num_idxs_reg=T,
                elem_size=VPAD,
            )
            nc.vector.tensor_add(acc[:], acc[:], gt[:])
            if debug_out and c == 0:
                nc.sync.dma_start(dg_d[:], gt[:].rearrange("p a b -> p (a b)"))
                nc.sync.dma_start(dwr_d[:], wrapped[:])

        # ---- outputs ----
        nc.sync.dma_start(acc_d[:], acc[:].rearrange("p a b -> p (a b)"))
        nc.sync.dma_start(idx_d[:], idxall[:, 0, :])

    return nc


# --------------------------------------------------------------------------
# host side
# --------------------------------------------------------------------------
def _split16(a32, scale):
    s = (a32 * np.float32(scale)).astype(np.float32)
    hi = s.astype(np.float16)
    lo = (s - hi.astype(np.float32)).astype(np.float16)
    return hi, lo


def prep_inputs(embeddings, rand_proj, codebook, values, cloc=CLOC, ncores=NCORES):
    """Full inputs -> list of per-core input dicts (+ nothing else)."""
    emb = np.ascontiguousarray(embeddings, dtype=np.float32).reshape(T, D)
    P = np.ascontiguousarray(rand_proj, dtype=np.float32)
    CB = np.ascontiguousarray(codebook, dtype=np.float32)
    VA = np.ascontiguousarray(values, dtype=np.float32)

    # X^T chunk-major: [128, KCH, T]
    xt = emb.T.reshape(KCH, 128, T).transpose(1, 0, 2)
    xh, xl = _split16(xt, S1)
    xh = np.ascontiguousarray(xh.reshape(128, KCH * T))
    xl = np.ascontiguousarray(xl.reshape(128, KCH * T))

    nuse = cloc * ncores
    # P: [c, D, E] -> [c, 128, KCH*128]
    pr = P[:nuse].reshape(nuse, KCH, 128, E).transpose(0, 2, 1, 3)
    ph, pl = _split16(pr, S2)
    ph = np.ascontiguousarray(ph.reshape(nuse, 128, KCH * E))
    pl = np.ascontiguousarray(pl.reshape(nuse, 128, KCH * E))

    # codebook transposed: [c, E, M]
    cbt = np.ascontiguousarray(CB[:nuse].transpose(0, 2, 1))
    ch, cl = _split16(cbt, 1.0)

    # e_sq in fp64 -> fp32, scaled by -S3/2, fp16 pair, [c, 2, M]
    esq = (CB[:nuse].astype(np.float64) ** 2).sum(axis=2).astype(np.float32)
    eq32 = (np.float32(-0.5 * S3) * esq).astype(np.float32)
    eh = eq32.astype(np.float16)
    el = (eq32 - eh.astype(np.float32)).astype(np.float16)
    eq = np.ascontiguousarray(np.stack([eh, el], axis=1))  # [c, 2, M]

    # padded values rows
    vp = np.zeros((nuse, M, VPAD), dtype=np.float32)
    vp[:, :, :V] = VA[:nuse]

    in_maps = []
    for r in range(ncores):
        cs = slice(r * cloc, (r + 1) * cloc)
        in_maps.append({
            "xh": xh, "xl": xl,
            "ph": np.ascontiguousarray(ph[cs]),
            "pl": np.ascontiguousarray(pl[cs]),
            "ch": np.ascontiguousarray(ch[cs]),
            "cl": np.ascontiguousarray(cl[cs]),
            "eq": np.ascontiguousarray(eq[cs]),
            "vl": np.ascontiguousarray(vp[cs].reshape(cloc * M, VPAD)),
        })
    return in_maps


def token_of_slot():
    """acc[pp, sl] holds gather row i = sl*128+pp = j*16+q (j = s*2+tc):
    token t = tc*128 + q*8 + s with q = pp%16, tc = (pp//16)%2, s = pp//32 + 4*sl."""
    tmap = np.zeros((128, 2), dtype=np.int64)
    for pp in range(128):
        q = pp % 16
        tcn = (pp // 16) % 2
        for sl in range(2):
            s = pp // 32 + 4 * sl
            tmap[pp, sl] = tcn * 128 + q * 8 + s
    return tmap


def combine_results(results, ncores=NCORES):
    tmap = token_of_slot()
    out = np.zeros((T, V), dtype=np.float32)
    for r in range(ncores):
        a = np.asarray(results[r]["acc"]).reshape(128, 2, VPAD)
        for tcn in range(2):
            out[tmap[:, tcn]] += a[:, tcn, :V]
    return (out / np.float32(C)).reshape(B, N, V)


def kernel(embeddings, rand_proj, codebook, values):
    if "nc" not in _CACHE:
        nc = build_nc()
        nc.finalize()
        _CACHE["nc"] = nc
    nc = _CACHE["nc"]
    in_maps = prep_inputs(embeddings, rand_proj, codebook, values)
    from concourse.bass_utils import run_bass_kernel_spmd
    res = run_bass_kernel_spmd(nc, in_maps, list(range(NCORES)))
    return combine_results(res.results)


# revision 27
# speedup vs baseline: 1.1478x; 1.1478x over previous
"""DKVB vq_codebook kernel for 8 Trainium2 NeuronCores.

Strategy (sharding_hint): shard the C=256 codebooks across 8 cores (32 each).
Each core computes, for its codebooks c and all 256 tokens t:
    xp   = X @ P_c                      (projection,  fp16 hi/lo 3-term on PE)
    s    = xp @ cb_c^T - 0.5*||cb_c||^2 (score; argmax s == argmin d2)
    idx  = argmax_m s                   (DVE max8 + max_index, exact fp32)
    out += values_c[idx]                (HBM dma_gather by row)
Host sums the 8 partial [256,10] results and divides by 256.

Numerics: fp16 hi/lo splits make every matmul term exact-to-~2^-22; the
score is assembled in fp32 PSUM, so the argmax matches the fp32 reference
(0 flips verified on the actual seed-0 data vs fp64 ground truth).

All tensors are host-pre-laid-out so every big DMA is partition-contiguous.
"""

import sys
import numpy as np

sys.path.insert(0, "/opt/trn_rl_repo")

B, N, D = 4, 64, 2048
C, M, E, V = 256, 4096, 128, 10
NCORES = 8
CLOC = C // NCORES          # codebooks per core
T = B * N                   # 256 tokens
KCH = D // 128              # 16 k-chunks for the projection matmul
MCH = M // 512              # 8 m-chunks of 512 for the score matmul
S1, S2, S3 = 32.0, 512.0, 128.0   # X scale, P scale, xp rescale target
VPAD = 64                   # values rows padded to 64 fp32 = 256B (dma_gather)

_CACHE = {}


# --------------------------------------------------------------------------
# device program
# --------------------------------------------------------------------------
def build_nc(cloc=CLOC, debug_out=False):
    import concourse.bacc as bacc
    import concourse.bass as bass
    import concourse.tile as tile
    from concourse import mybir
    from contextlib import ExitStack

    f16 = mybir.dt.float16
    f32 = mybir.dt.float32
    u16 = mybir.dt.uint16
    AF = mybir.ActivationFunctionType

    nc = bacc.Bacc("TRN2", target_bir_lowering=False, debug=True)

    # ---- I/O ----
    xh_d = nc.dram_tensor("xh", [128, KCH * T], f16, kind="ExternalInput")
    xl_d = nc.dram_tensor("xl", [128, KCH * T], f16, kind="ExternalInput")
    ph_d = nc.dram_tensor("ph", [cloc, 128, KCH * 128], f16, kind="ExternalInput")
    pl_d = nc.dram_tensor("pl", [cloc, 128, KCH * 128], f16, kind="ExternalInput")
    ch_d = nc.dram_tensor("ch", [cloc, 128, M], f16, kind="ExternalInput")
    cl_d = nc.dram_tensor("cl", [cloc, 128, M], f16, kind="ExternalInput")
    eq_d = nc.dram_tensor("eq", [cloc, 2, M], f16, kind="ExternalInput")
    vl_d = nc.dram_tensor("vl", [cloc * M, VPAD], f32, kind="ExternalInput")

    acc_d = nc.dram_tensor("acc", [128, 2 * VPAD], f32, kind="ExternalOutput")
    idx_d = nc.dram_tensor("idx", [128, 2 * cloc], u16, kind="ExternalOutput")
    if debug_out:
        dwr_d = nc.dram_tensor("dbg_wr", [128, 16], u16, kind="ExternalOutput")
        dg_d = nc.dram_tensor("dbg_g", [128, 2 * VPAD], f32, kind="ExternalOutput")

    with tile.TileContext(nc) as tc, ExitStack() as ctx:
        p_x = ctx.enter_context(tc.tile_pool(name="x", bufs=1))
        p_p = ctx.enter_context(tc.tile_pool(name="p", bufs=2))
        p_cb = ctx.enter_context(tc.tile_pool(name="cb", bufs=2))
        p_eq = ctx.enter_context(tc.tile_pool(name="eq", bufs=2))
        p_xp16 = ctx.enter_context(tc.tile_pool(name="xp16", bufs=2))
        p_score = ctx.enter_context(tc.tile_pool(name="score", bufs=3))
        p_mx = ctx.enter_context(tc.tile_pool(name="mx", bufs=4))
        p_misc = ctx.enter_context(tc.tile_pool(name="misc", bufs=1))
        p_g = ctx.enter_context(tc.tile_pool(name="g", bufs=2))
        p_psxp = ctx.enter_context(tc.tile_pool(name="psxp", bufs=2, space="PSUM"))
        p_psdot = ctx.enter_context(tc.tile_pool(name="psdot", bufs=3, space="PSUM"))
        p_dram = ctx.enter_context(tc.tile_pool(name="scratch", bufs=2, space="DRAM"))

        # ---- static tiles ----
        x_h = p_x.tile([128, KCH * T], f16, tag="xh")
        x_l = p_x.tile([128, KCH * T], f16, tag="xl")
        nc.sync.dma_start(x_h[:], xh_d[:])
        nc.sync.dma_start(x_l[:], xl_d[:])

        ones16 = p_misc.tile([2, 128], f16, tag="ones")
        nc.vector.memset(ones16[:], 1.0)

        idxall = p_misc.tile([128, 8, 2 * cloc], u16, tag="idxall")
        acc = p_misc.tile([128, 2, VPAD], f32, tag="acc")
        nc.vector.memset(acc[:], 0.0)

        def load_cb_weights(c):
            p_h = p_p.tile([128, KCH * 128], f16, tag="ph")
            p_l = p_p.tile([128, KCH * 128], f16, tag="pl")
            nc.sync.dma_start(p_h[:], ph_d[c])
            nc.sync.dma_start(p_l[:], pl_d[c])
            return p_h, p_l

        def compute_xp(p_h, p_l):
            """projection xp_ps[e, t] = sum_d P[d,e] * X[d,t] (scaled), then
            split to an fp16 pair at scale S3."""
            xp_ps = p_psxp.tile([128, T], f32, tag="xp")
            n3 = 3 * KCH
            i = 0
            for k in range(KCH):
                lw_h = p_h[:, k * 128:(k + 1) * 128]
                lw_l = p_l[:, k * 128:(k + 1) * 128]
                rh_h = x_h[:, k * T:(k + 1) * T]
                rh_l = x_l[:, k * T:(k + 1) * T]
                nc.tensor.matmul(xp_ps[:], lw_h, rh_h, start=(i == 0), stop=False)
                i += 1
                nc.tensor.matmul(xp_ps[:], lw_h, rh_l, start=False, stop=False)
                i += 1
                nc.tensor.matmul(xp_ps[:], lw_l, rh_h, start=False, stop=(i == n3 - 1))
                i += 1
            sc = float(S3 / (S1 * S2))
            xh16 = p_xp16.tile([128, T], f16, tag="xh16")
            xp32 = p_xp16.tile([128, T], f32, tag="xp32")
            nc.scalar.activation(xh16[:], xp_ps[:], AF.Copy, scale=sc)
            nc.scalar.activation(xp32[:], xp_ps[:], AF.Copy, scale=sc)
            xl16 = p_xp16.tile([128, T], f16, tag="xl16")
            nc.vector.tensor_sub(xl16[:], xp32[:], xh16[:])
            return xh16, xl16

        pw = load_cb_weights(0)
        xpair = compute_xp(*pw)
        for c in range(cloc):
            # ---- load per-codebook tables ----
            cb_h = p_cb.tile([128, M], f16, tag="ch")
            cb_l = p_cb.tile([128, M], f16, tag="cl")
            nc.sync.dma_start(cb_h[:], ch_d[c])
            nc.sync.dma_start(cb_l[:], cl_d[c])
            eq_t = p_eq.tile([2, M], f16, tag="eq")
            nc.sync.dma_start(eq_t[:], eq_d[c])
            xh16, xl16 = xpair

            # ---- score matmuls + scan per token-chunk ----
            # weights-major emission within each m-half so the PE reuses the
            # stationary operand across consecutive matmuls.
            for tcn in range(2):
                lw_xh = xh16[:, tcn * 128:(tcn + 1) * 128]
                lw_xl = xl16[:, tcn * 128:(tcn + 1) * 128]
                score = p_score.tile([128, M], f32, tag="score")
                for quad in range(MCH // 2):
                    ds = p_psdot.tile([128, 1024], f32, tag="ds")
                    # (bank slice of ds, m-range of cb/score)
                    banks = [
                        (ds[:, b * 512:(b + 1) * 512],
                         slice((2 * quad + b) * 512, (2 * quad + b + 1) * 512))
                        for b in range(2)
                    ]
                    for db, ms in banks:
                        nc.tensor.matmul(db, lw_xh, cb_h[:, ms], start=True, stop=False)
                    for db, ms in banks:
                        nc.tensor.matmul(db, lw_xh, cb_l[:, ms], start=False, stop=False)
                    for db, ms in banks:
                        nc.tensor.matmul(db, lw_xl, cb_h[:, ms], start=False, stop=False)
                    for db, ms in banks:
                        nc.tensor.matmul(db, ones16[:], eq_t[:, ms], start=False, stop=True)
                    nc.scalar.activation(
                        score[:, quad * 1024:(quad + 1) * 1024], ds[:], AF.Copy)

                mx = p_mx.tile([128, 8], f32, tag="mx")
                nc.vector.max(mx[:], score[:])
                nc.vector.max_index(idxall[:, :, 2 * c + tcn], mx[:], score[:])

                # software pipeline: give the PE the next codebook's
                # projection while ScalarE/DVE digest this token-chunk.
                if tcn == 0 and c + 1 < cloc:
                    pw = load_cb_weights(c + 1)
                    xpair_next = compute_xp(*pw)

            if c + 1 < cloc:
                xpair = xpair_next

            # ---- idx round-trip through DRAM to the wrapped gather layout ----
            # gather slot i = j*16 + q with j = s*2 + tc reads token
            # t = tc*128 + q*8 + s; gathered row i lands on out partition i%128.
            idq = p_dram.tile([128, 2], u16, tag="idq")
            nc.sync.dma_start(idq[:], idxall[:, 0, 2 * c:2 * c + 2])
            wrapped = p_g.tile([128, 16], u16, tag="wrapped")
            src = idq[:].rearrange("(q s) tc -> q s tc", s=8)
            for g in range(8):
                nc.sync.dma_start(wrapped[16 * g:16 * (g + 1), :], src)
            gt = p_g.tile([128, 2, VPAD], f32, tag="g")
            nc.gpsimd.dma_gather(
                gt[:],
                vl_d[c * M:(c + 1) * M, :],
                wrapped[:].bitcast(mybir.dt.int16),
                num_idxs=T,
                num_idxs_reg=T,
                elem_size=VPAD,
            )
            nc.vector.tensor_add(acc[:], acc[:], gt[:])
            if debug_out and c == 0:
                nc.sync.dma_start(dg_d[:], gt[:].rearrange("p a b -> p (a b)"))
                nc.sync.dma_start(dwr_d[:], wrapped[:])

        # ---- outputs ----
        nc.sync.dma_start(acc_d[:], acc[:].rearrange("p a b -> p (a b)"))
        nc.sync.dma_start(idx_d[:], idxall[:, 0, :])

    return nc


# --------------------------------------------------------------------------
# host side
# --------------------------------------------------------------------------
def _split16(a32, scale):
    s = (a32 * np.float32(scale)).astype(np.float32)
    hi = s.astype(np.float16)
    lo = (s - hi.astype(np.float32)).astype(np.float16)
    return hi, lo


def prep_inputs(embeddings, rand_proj, codebook, values, cloc=CLOC, ncores=NCORES):
    """Full inputs -> list of per-core input dicts (+ nothing else)."""
    emb = np.ascontiguousarray(embeddings, dtype=np.float32).reshape(T, D)
    P = np.ascontiguousarray(rand_proj, dtype=np.float32)
    CB = np.ascontiguousarray(codebook, dtype=np.float32)
    VA = np.ascontiguousarray(values, dtype=np.float32)

    # X^T chunk-major: [128, KCH, T]
    xt = emb.T.reshape(KCH, 128, T).transpose(1, 0, 2)
    xh, xl = _split16(xt, S1)
    xh = np.ascontiguousarray(xh.reshape(128, KCH * T))
    xl = np.ascontiguousarray(xl.reshape(128, KCH * T))

    nuse = cloc * ncores
    # P: [c, D, E] -> [c, 128, KCH*128]
    pr = P[:nuse].reshape(nuse, KCH, 128, E).transpose(0, 2, 1, 3)
    ph, pl = _split16(pr, S2)
    ph = np.ascontiguousarray(ph.reshape(nuse, 128, KCH * E))
    pl = np.ascontiguousarray(pl.reshape(nuse, 128, KCH * E))

    # codebook transposed: [c, E, M]
    cbt = np.ascontiguousarray(CB[:nuse].transpose(0, 2, 1))
    ch, cl = _split16(cbt, 1.0)

    # e_sq in fp64 -> fp32, scaled by -S3/2, fp16 pair, [c, 2, M]
    esq = (CB[:nuse].astype(np.float64) ** 2).sum(axis=2).astype(np.float32)
    eq32 = (np.float32(-0.5 * S3) * esq).astype(np.float32)
    eh = eq32.astype(np.float16)
    el = (eq32 - eh.astype(np.float32)).astype(np.float16)
    eq = np.ascontiguousarray(np.stack([eh, el], axis=1))  # [c, 2, M]

    # padded values rows
    vp = np.zeros((nuse, M, VPAD), dtype=np.float32)
    vp[:, :, :V] = VA[:nuse]

    in_maps = []
    for r in range(ncores):
        cs = slice(r * cloc, (r + 1) * cloc)
        in_maps.append({
            "xh": xh, "xl": xl,
            "ph": np.ascontiguousarray(ph[cs]),
            "pl": np.ascontiguousarray(pl[cs]),
            "ch": np.ascontiguousarray(ch[cs]),
            "cl": np.ascontiguousarray(cl[cs]),
            "eq": np.ascontiguousarray(eq[cs]),
            "vl": np.ascontiguousarray(vp[cs].reshape(cloc * M, VPAD)),
        })
    return in_maps


def token_of_slot():
    """acc[pp, sl] holds gather row i = sl*128+pp = j*16+q (j = s*2+tc):
    token t = tc*128 + q*8 + s with q = pp%16, tc = (pp//16)%2, s = pp//32 + 4*sl."""
    tmap = np.zeros((128, 2), dtype=np.int64)
    for pp in range(128):
        q = pp % 16
        tcn = (pp // 16) % 2
        for sl in range(2):
            s = pp // 32 + 4 * sl
            tmap[pp, sl] = tcn * 128 + q * 8 + s
    return tmap


def combine_results(results, ncores=NCORES):
    tmap = token_of_slot()
    out = np.zeros((T, V), dtype=np.float32)
    for r in range(ncores):
        a = np.asarray(results[r]["acc"]).reshape(128, 2, VPAD)
        for tcn in range(2):
            out[tmap[:, tcn]] += a[:, tcn, :V]
    return (out / np.float32(C)).reshape(B, N, V)


def kernel(embeddings, rand_proj, codebook, values):
    if "nc" not in _CACHE:
        nc = build_nc()
        nc.finalize()
        _CACHE["nc"] = nc
    nc = _CACHE["nc"]
    in_maps = prep_inputs(embeddings, rand_proj, codebook, values)
    from concourse.bass_utils import run_bass_kernel_spmd
    res = run_bass_kernel_spmd(nc, in_maps, list(range(NCORES)))
    return combine_results(res.results)
